# revision 9
# baseline (speedup 1.0000x reference)
"""MultiHeadDistanceLayer kernel for Trainium2, 8 NeuronCores.

Reference math (per batch b, head h, L=2048, F=512, H=8, d=64):
    qk = inputs[b].T + pos          # (L, F)
    q  = qk @ Wq.T + bq             # (L, H*d) -> per-head (L, d)
    k  = qk @ Wk.T + bk
    S  = q_h @ k_h.T / sqrt(d)      # (L, L)
    attn = softmax(S, axis=-1)
    dist_h = attn @ k_h             # (L, d)   (reference uses k as values)
    outputs: dist (B, L, H*d), attn_out (B, L, L, H)

Sharding: 16 (b, h) pairs over 8 cores -> core c handles b = c // 4 and
heads {2*(c%4), 2*(c%4)+1}. Each core computes its two heads' full LxL
attention locally in a transposed layout (k-positions on partitions,
q-positions on the free axis), which makes the second matmul
(dist = attn @ k) need no on-chip transpose, and yields row-sums for free
via a ones-column appended to the k operand. Host re-interleaves the
per-head transposed outputs into the final (B, L, L, H) / (B, L, H*d)
arrays (that permute is required regardless of device layout because H is
the innermost output axis).

All matmuls run as float32r (fp32 data rounded to ~12 mantissa bits on the
producing instruction; PE accumulates in full fp32).
"""

import sys
import numpy as np

for _p in ("/opt/trn_rl_repo", "/root/.axon_site/_ro/trn_rl_repo"):
    if _p not in sys.path:
        sys.path.append(_p)

import concourse.bass as bass
import concourse.mybir as mybir
import concourse.tile as tile
from concourse.bass_utils import run_bass_kernel_spmd
from concourse.masks import make_identity

F32 = mybir.dt.float32
F32R = mybir.dt.float32r
AF = mybir.ActivationFunctionType

B, F, L = 2, 512, 2048
H, D = 8, 64
TEMPERATURE = 10000.0
NCORES = 8
FC = F // 128          # 4 f-chunks
KC = L // 128          # 16 k-chunks
QH = 2                 # q halves
QHW = L // QH          # 1024 wide


def _positional_encoding_t():
    """posT (F, L) float32, matching reference.positional_encoding."""
    embed = np.arange(L, dtype=np.float32)
    dim_t = np.arange(F, dtype=np.float32)
    dim_t = (TEMPERATURE ** (2.0 * np.floor(dim_t / 2.0) / F)).astype(np.float32)
    pos = embed[:, None] / dim_t[None, :]              # (L, F)
    out = np.empty((L, F), dtype=np.float32)
    out[:, 0::2] = np.sin(pos[:, 0::2])
    out[:, 1::2] = np.cos(pos[:, 1::2])
    return np.ascontiguousarray(out.T)                 # (F, L)


_SPLIT_EXEMPT = ("InstAllEngineBarrier", "InstUnconditionalBranch",
                 "InstCompareAndBranch", "InstHalt", "InstEventSemaphore")


def _split_multi_waits(nc):
    """walrus codegen on this toolchain accepts a single sync-wait per
    instruction for several forms (fused 4-byte matmul, HWDGE direct DMA,
    drain). Hoist extra waits onto same-engine NoOps just before."""
    cnt = 0
    for fn in nc.m.functions:
        for bb in fn.blocks:
            out = []
            for ins in bb.instructions:
                si = getattr(ins, "sync_info", None)
                if (si is not None and si.on_wait and len(si.on_wait) > 1
                        and type(ins).__name__ not in _SPLIT_EXEMPT):
                    waits = list(si.on_wait)
                    for w in waits[:-1]:
                        out.append(mybir.InstNoOp(
                            name=f"{ins.name}-wsplit-{cnt}", engine=ins.engine,
                            sync_info=mybir.SyncInfo(on_wait=[w], on_update=[])))
                        cnt += 1
                    ins.sync_info = mybir.SyncInfo(on_wait=[waits[-1]],
                                                   on_update=list(si.on_update))
                out.append(ins)
            bb.instructions = out
    return cnt


def build_nc():
    import os
    stage = int(os.environ.get("KBUILD_STAGE", "5"))
    nc = bass.Bass()
    x_d = nc.declare_dram_parameter("x", [F, L], F32, isOutput=False)
    pos_d = nc.declare_dram_parameter("post", [F, L], F32, isOutput=False)
    wqt_d = nc.declare_dram_parameter("wqt", [F, 128], F32, isOutput=False)
    wkt_d = nc.declare_dram_parameter("wkt", [F, 128], F32, isOutput=False)
    bq_d = nc.declare_dram_parameter("bq", [1, 128], F32, isOutput=False)
    bk_d = nc.declare_dram_parameter("bk", [1, 128], F32, isOutput=False)
    attn_d = nc.declare_dram_parameter("attn_t", [2, L, L], F32, isOutput=True)
    dist_d = nc.declare_dram_parameter("dist_t", [2, D, L], F32, isOutput=True)

    with tile.TileContext(nc) as tc:
        with tc.tile_pool(name="const", bufs=1) as const, \
             tc.tile_pool(name="tmp", bufs=1) as tmp, \
             tc.tile_pool(name="expst", bufs=17) as expst, \
             tc.tile_pool(name="attnout", bufs=4) as attnout, \
             tc.tile_pool(name="small", bufs=2) as small, \
             tc.tile_pool(name="psum", bufs=1, space="PSUM") as psum:

            # ---- constants / inputs -------------------------------------
            ones_f = tmp.tile([1, L], F32, tag="onesf")
            nc.vector.memset(ones_f, 1.0)
            ones_row = const.tile([1, L], F32R, tag="ones")
            nc.vector.tensor_copy(ones_row, ones_f)

            ident_f = const.tile([128, 128], F32, tag="identf")
            make_identity(nc, ident_f)
            ident = const.tile([128, 128], F32R, tag="ident")
            nc.vector.tensor_copy(ident, ident_f)

            # qk = x + posT, rounded to f32r; 4 chunks of [128, L]
            qk = []
            for c in range(FC):
                xt = tmp.tile([128, L], F32, tag="xload")
                pt = tmp.tile([128, L], F32, tag="pload")
                nc.sync.dma_start(out=xt, in_=x_d[128 * c:128 * (c + 1), :])
                nc.sync.dma_start(out=pt, in_=pos_d[128 * c:128 * (c + 1), :])
                q = const.tile([128, L], F32R, tag=f"qk{c}")
                nc.vector.tensor_add(q, xt, pt)
                qk.append(q)

            # weights (transposed, f-major) + biases, rounded to f32r
            wq, wk = [], []
            for c in range(FC):
                for (dram, lst, tg) in ((wqt_d, wq, "wq"), (wkt_d, wk, "wk")):
                    wt = tmp.tile([128, 128], F32, tag="wload")
                    nc.sync.dma_start(out=wt, in_=dram[128 * c:128 * (c + 1), :])
                    w = const.tile([128, 128], F32R, tag=f"{tg}{c}")
                    nc.vector.tensor_copy(w, wt)
                    lst.append(w)
            bq_t = tmp.tile([1, 128], F32, tag="bload")
            bk_t = tmp.tile([1, 128], F32, tag="bload")
            nc.sync.dma_start(out=bq_t, in_=bq_d[:, :])
            nc.sync.dma_start(out=bk_t, in_=bk_d[:, :])
            bq = const.tile([1, 128], F32R, tag="bq")
            bk = const.tile([1, 128], F32R, tag="bk")
            nc.vector.tensor_copy(bq, bq_t)
            nc.vector.tensor_copy(bk, bk_t)

            # ---- projections: qT2/kT2 [128 (2 heads x d), L] ------------
            qT2 = const.tile([128, L], F32R, tag="qT2")
            kT2 = const.tile([128, L], F32R, tag="kT2")
            for (w, b, out_t) in ((wq, bq, qT2), (wk, bk, kT2)):
                for n in range(L // 512):
                    ps = psum.tile([128, 1024], F32, tag="sp")
                    for c in range(FC):
                        nc.tensor.matmul(ps[:, 0:512], w[c],
                                         qk[c][:, 512 * n:512 * (n + 1)],
                                         start=(c == 0), stop=False)
                    nc.tensor.matmul(ps[:, 0:512], b, ones_row[:, 512 * n:512 * (n + 1)],
                                     start=False, stop=True)
                    nc.vector.tensor_copy(out_t[:, 512 * n:512 * (n + 1)],
                                          ps[:, 0:512])

            # ---- k_aug per head: [128, KC, 65]; col 64 = ones -----------
            ka_init = tmp.tile([128, KC * 65], F32, tag="kainit")
            nc.vector.memset(ka_init, 1.0)
            kaug = []
            for p in range(2):
                ka = const.tile([128, KC, 65], F32R, tag=f"kaug{p}")
                nc.vector.tensor_copy(ka.rearrange("p a b -> p (a b)"), ka_init)
                for j in range(KC):
                    pt = psum.tile([128, 1024], F32, tag="dp")
                    nc.tensor.transpose(pt[:, 0:64].bitcast(F32R),
                                        kT2[64 * p:64 * (p + 1), 128 * j:128 * (j + 1)],
                                        ident[64 * p:64 * (p + 1), 64 * p:64 * (p + 1)])
                    nc.vector.tensor_copy(ka[:, j, 0:64], pt[:, 0:64])
                kaug.append(ka)

            # ---- main blocks: per head, per q-half ----------------------
            for p in range(2):
                if stage < 1:
                    break
                for qh in range(QH):
                    q0 = qh * QHW
                    dp = psum.tile([65, 1024], F32, tag="dp")
                    ets = []
                    for j in range(KC):
                        sp = psum.tile([128, 1024], F32, tag="sp")
                        for n in range(2):
                            nc.tensor.matmul(
                                sp[:, 512 * n:512 * (n + 1)],
                                kT2[64 * p:64 * (p + 1), 128 * j:128 * (j + 1)],
                                qT2[64 * p:64 * (p + 1), q0 + 512 * n:q0 + 512 * (n + 1)],
                                start=True, stop=True)
                        et = expst.tile([128, QHW], F32R, tag="et")
                        nc.scalar.activation(out=et, in_=sp, func=AF.Exp,
                                             scale=float(D) ** -0.5)
                        if stage >= 2:
                            for n in range(2):
                                nc.tensor.matmul(dp[:, 512 * n:512 * (n + 1)],
                                                 kaug[p][:, j, :],
                                                 et[:, 512 * n:512 * (n + 1)],
                                                 start=(j == 0), stop=(j == KC - 1))
                        ets.append(et)
                    if stage < 3:
                        continue

                    # row-sums -> broadcast reciprocal [128, QHW] in SBUF
                    rs = small.tile([1, QHW], F32R, tag="rs")
                    nc.vector.tensor_copy(rs, dp[64:65, :])
                    rb_ps = psum.tile([128, 1024], F32, tag="sp")
                    for n in range(2):
                        nc.tensor.matmul(rb_ps[:, 512 * n:512 * (n + 1)],
                                         ones_row[:, 0:128],
                                         rs[:, 512 * n:512 * (n + 1)],
                                         start=True, stop=True)
                    rb = small.tile([128, QHW], F32, tag="rb")
                    nc.vector.reciprocal(rb, rb_ps)
                    if stage < 4:
                        continue

                    # normalize + store attention (transposed per head)
                    for j in range(KC):
                        at = attnout.tile([128, QHW], F32, tag="at")
                        nc.vector.tensor_mul(at, ets[j].bitcast(F32), rb)
                        nc.sync.dma_start(
                            out=attn_d[p, 128 * j:128 * (j + 1), q0:q0 + QHW],
                            in_=at)

                    if stage < 5:
                        continue
                    # normalize + store dist (transposed per head)
                    dn = small.tile([D, QHW], F32, tag="dn")
                    nc.vector.tensor_mul(dn, dp[0:D, :], rb[0:D, :])
                    nc.sync.dma_start(out=dist_d[p, :, q0:q0 + QHW], in_=dn)

    _split_multi_waits(nc)
    return nc


_NC_CACHE = None


def _get_nc():
    global _NC_CACHE
    if _NC_CACHE is None:
        _NC_CACHE = build_nc()
    return _NC_CACHE


def run(inputs, Wq, bq, Wk, bk, Wv, bv, trace=False, **spmd_kwargs):
    """Returns ((dist, attn_out), BassKernelResults)."""
    del Wv, bv  # unused by the reference (original-model bug kept faithfully)
    inputs = np.ascontiguousarray(np.asarray(inputs, dtype=np.float32))
    post = _positional_encoding_t()
    in_maps = []
    for c in range(NCORES):
        b = c // 4
        hh = c % 4
        rows = slice(128 * hh, 128 * (hh + 1))
        in_maps.append({
            "x": inputs[b],
            "post": post,
            "wqt": np.ascontiguousarray(np.asarray(Wq, np.float32)[rows].T),
            "wkt": np.ascontiguousarray(np.asarray(Wk, np.float32)[rows].T),
            "bq": np.asarray(bq, np.float32)[None, rows],
            "bk": np.asarray(bk, np.float32)[None, rows],
        })
    res = run_bass_kernel_spmd(_get_nc(), in_maps, list(range(NCORES)),
                               trace=trace, **spmd_kwargs)

    attn_out = np.empty((B, L, L, H), dtype=np.float32)
    dist = np.empty((B, L, H * D), dtype=np.float32)
    for c in range(NCORES):
        b = c // 4
        r = res.results[c]
        for j in range(2):
            h = 2 * (c % 4) + j
            attn_out[b, :, :, h] = r["attn_t"][j].T
            dist[b, :, D * h:D * (h + 1)] = r["dist_t"][j].T
    return (dist, attn_out), res


def kernel(inputs, Wq, bq, Wk, bk, Wv, bv):
    out, _ = run(inputs, Wq, bq, Wk, bk, Wv, bv)
    return out
